# revision 1
# baseline (speedup 1.0000x reference)
"""BiLSTM tagger (B=32, S=256, E=H=512, V=50000, T=64) on 8 Trainium2 cores.

Strategy (single SPMD launch, all 8 cores distinct work):
  - Direction split: cores 0-3 forward, 4-7 backward (direction comes only
    from per-core input data: backward cores get time-reversed xsT).
  - Batch split: each core handles Bc = 8 of the 32 batch elements.
    Core i (fwd) pairs with core i+4 (bwd) on the same batch slice;
    replica groups [[0,4],[1,5],[2,6],[3,7]].
  - Per layer the sequential LSTM recurrence runs as a 256-step scan in
    "chunked" layout: psum [128, 16m*8b] holds the gates [f|i|o|ch],
    elem (p, m*8+b) = gate-dim 128m+p, batch b.  The x-projection value
    gx is seeded into PSUM by an identity matmul (start=True) so no
    vector add sits on the critical recurrence chain.
  - h state lives in a block ring laid out (u, k, b); each 8-step block
    is flushed with one contiguous DMA to hout and one u-reversed DMA
    that feeds a pairwise AllGather into canonical slot 31-blk, so the
    gathered peer sequence reads time-aligned with no reversal at use.
  - gx2 (= W2.T @ [own h1; bias; peer h1]) runs as 8 four-block GEMM
    passes (N=256) interleaved into scan1 as soon as the own blocks and
    peer slots are available; the per-core packed weights zero the own
    half of the gathered buffer, making the fused GEMM exact.  The tag
    projection runs the same way inside scan2.
  - All streaming DMAs issue from the GPSIMD queue (cheap sequencer).

Numerics: matmuls bf16 with fp32 PSUM accumulation; gx stored bf16;
gate activations and cell state fp32; h stored bf16.  Tag bias is added
on host.
"""

import numpy as np
import ml_dtypes
import concourse.bass as bass
import concourse.bacc as bacc
import concourse.mybir as mybir
from concourse.tile import TileContext
from concourse.bass_utils import run_bass_kernel_spmd

F32 = mybir.dt.float32
BF16 = mybir.dt.bfloat16
AF = mybir.ActivationFunctionType

S, B, E, H, V, T = 256, 32, 512, 512, 50000, 64
Bc = 8                      # per-core batch
N = S * Bc                  # columns per core (col = t*Bc + b)
BLK = 8                     # steps per block
NB = S // BLK               # 32 blocks
BC4 = 4 * Bc                # one gate-group width in (k,b) layout
UC = 16 * Bc                # per-step gate columns (16 m-blocks x Bc)
BW = BLK * BC4              # block width in h cols (u k b) = 256
PB = 4                      # blocks per GEMM pass
NPASS = NB // PB            # 8 passes
NC = PB * BLK * Bc          # pass column count = 256
REPLICA_GROUPS = [[0, 4], [1, 5], [2, 6], [3, 7]]
GATE_PERM = [2, 0, 1, 3]    # reference gate order [f,i,c,o] -> ours [ch,f,i,o]

ts = lambda i, n: slice(i * n, (i + 1) * n)

FP8_H = True          # fp8e4m3 DoubleRow recurrence (halves PE instr count)
FP8 = mybir.dt.float8e4
NPFP8 = mybir.dt.np(FP8)
W_SCALE = 32.0        # recurrent weights stored *32 in fp8
H_SCALE = 8.0         # h stored *8 in fp8
G_SCALE = W_SCALE * H_SCALE   # psum gates carry x256; undone in activation


def _scan_cell(nc, tc, pools, wh_in, gx_dram, hout_dram, hrev_tiles, ag_tiles,
               block_cb):
    """LSTM scan over S steps in NB blocks.

    gx_dram  [128, 16*N] bf16 (p, (m s b)): x-projection (+bias), read per
             block into contiguous (m, u, b) tiles.
    hout_dram [128, S*BC4] bf16 (p, (s k b)): time-aligned h output.
    hrev_tiles [128, BW] / ag_tiles [256, BW]: per-block exchange; block
             blk's gather lands in ag_tiles[NB-1-blk] (canonical slot).
    block_cb(blk): called after block blk's collective is issued.
    """
    wpool, gxpool, state, gbuf, hring, psum, ident, onesb = pools
    if FP8_H:
        w8 = wpool.tile([128, 4 * 2048], FP8, name="w8", tag="w8")
        nc.sync.dma_start(out=w8[:, :].rearrange("p (k m) -> p k m", k=4),
                          in_=wh_in.rearrange("(k p) m -> p k m", p=128))
        w8v = w8[:, :].rearrange("p (k m) -> p k m", k=4)
    else:
        wt = []
        for k in range(4):
            w = wpool.tile([128, 2048], BF16, name=f"wt{k}", tag=f"wt{k}")
            nc.sync.dma_start(out=w[:, :], in_=wh_in[128 * k:128 * (k + 1), :])
            wt.append(w)
    c = state.tile([128, BC4], F32, name="cst", tag="cst")
    h0 = state.tile([128, BC4], FP8 if FP8_H else BF16, name="h0", tag="h0")
    nc.vector.memset(c[:, :], 0.0)
    nc.vector.memset(h0[:, :], 0.0)

    gsrc = gx_dram.rearrange("p (m s b) -> p m s b", m=16, b=Bc)

    gxt = [None] * NB

    def fetch(blk):
        if blk >= NB:
            return
        t = gxpool.tile([128, BLK * UC], BF16, name="gx", tag="gx")
        nc.gpsimd.dma_start(out=t[:, :], in_=gsrc[:, :, ts(blk, BLK), :])
        gxt[blk] = t

    fetch(0)
    fetch(1)

    hprev = h0[:, :]
    for blk in range(NB):
        fetch(blk + 2)
        hb = hring.tile([128, BW], BF16, name="hb", tag="hb")
        gview = gxt[blk][:, :].rearrange("p (m u b) -> p m u b", m=16, b=Bc)
        for u in range(BLK):
            ps = psum.tile([128, UC], F32, name="ps", tag="ps")
            # seed psum with gx via identity matmul (independent of h, so it
            # fills the PE gap in the previous step's elementwise window)
            nc.tensor.matmul(ps[:, :], lhsT=ident[:, :], rhs=gview[:, :, u, :],
                             start=True, stop=False, skip_group_check=True)
            if FP8_H:
                h8v = hprev.rearrange("p (k b) -> p k b", k=4)
                for m in range(16):
                    for j in range(2):
                        nc.tensor.matmul(
                            ps[:, Bc * m:Bc * (m + 1)],
                            lhsT=w8v[:, 2 * j:2 * j + 2, ts(m, 128)],
                            rhs=h8v[:, 2 * j:2 * j + 2, :],
                            start=False, stop=(j == 1),
                            perf_mode=mybir.MatmulPerfMode.DoubleRow,
                            skip_group_check=True,
                        )
            else:
                for m in range(16):
                    for k in range(4):
                        nc.tensor.matmul(
                            ps[:, Bc * m:Bc * (m + 1)],
                            lhsT=wt[k][:, 128 * m:128 * (m + 1)],
                            rhs=hprev[:, Bc * k:Bc * (k + 1)],
                            start=False, stop=(k == 3),
                            skip_group_check=True,
                        )
            act = gbuf.tile([128, UC], F32, name="act", tag="act")
            t2 = gbuf.tile([128, BC4], F32, name="t2", tag="t2")
            tc_t = gbuf.tile([128, BC4], F32, name="tct", tag="tct")
            hslot = hb[:, ts(u, BC4)]
            asc = 1.0 / G_SCALE if FP8_H else 1.0
            # gates [ch | f | i | o]; ch weights carry x2 so sigmoid covers
            # everything: tanh(x) = 2*sigmoid(2x) - 1.  o is last in matmul
            # order, so sig_a starts while the o matmuls drain; sig_b (o)
            # has slack until the h multiply.
            nc.scalar.activation(act[:, 0:3 * BC4], ps[:, 0:3 * BC4],
                                 AF.Sigmoid, scale=asc)
            cht = gbuf.tile([128, BC4], F32, name="cht", tag="cht")
            nc.vector.scalar_tensor_tensor(
                out=cht[:, :], in0=act[:, 0:BC4], scalar=2.0,
                in1=onesb[:, :], op0=mybir.AluOpType.mult,
                op1=mybir.AluOpType.subtract)
            nc.scalar.activation(act[:, 3 * BC4:UC], ps[:, 3 * BC4:UC],
                                 AF.Sigmoid, scale=asc)
            nc.vector.tensor_mul(out=c[:, :], in0=act[:, BC4:2 * BC4],
                                 in1=c[:, :])
            nc.vector.tensor_mul(out=t2[:, :], in0=act[:, 2 * BC4:3 * BC4],
                                 in1=cht[:, :])
            nc.vector.tensor_add(out=c[:, :], in0=c[:, :], in1=t2[:, :])
            nc.scalar.activation(tc_t[:, :], c[:, :], AF.Tanh)
            if FP8_H:
                h8n = gbuf.tile([128, BC4], FP8, name="h8", tag="h8")
                nc.vector.scalar_tensor_tensor(
                    out=h8n[:, :], in0=act[:, 3 * BC4:UC], scalar=H_SCALE,
                    in1=tc_t[:, :], op0=mybir.AluOpType.mult,
                    op1=mybir.AluOpType.mult)
                nc.vector.tensor_mul(out=hslot, in0=act[:, 3 * BC4:UC],
                                     in1=tc_t[:, :])
                hprev = h8n[:, :]
            else:
                nc.vector.tensor_mul(out=hslot, in0=act[:, 3 * BC4:UC],
                                     in1=tc_t[:, :])
                hprev = hslot
        # hout block: hb (u k b) is exactly hout's (s k b) col order
        nc.gpsimd.dma_start(out=hout_dram[:, ts(blk, BW)], in_=hb[:, :])
        # u-reversed copy for the exchange (single DMA, negative u stride)
        hbase = hb[:, :]
        rsrc = bass.AP(hbase.tensor, hbase.offset + (BLK - 1) * BC4,
                       [hbase.ap[0], [-BC4, BLK], [1, BC4]])
        nc.gpsimd.dma_start(out=hrev_tiles[blk][:, :].rearrange(
            "p (u x) -> p u x", u=BLK), in_=rsrc)
        nc.gpsimd.collective_compute(
            "AllGather", mybir.AluOpType.bypass,
            replica_groups=REPLICA_GROUPS,
            ins=[hrev_tiles[blk].opt()], outs=[ag_tiles[NB - 1 - blk].opt()],
        )
        block_cb(blk)


def _load_pass_rhs(nc, rpool, tagp, hout, ag_tiles, p):
    """Load the own/gathered rhs tiles for pass p (blocks PB*p..PB*p+PB-1).

    own [128, 4*NC]: chunk k at ts(k, NC), cols (s, b) time-aligned.
    agt [128, 2*PB*BW]: layout r(2) x (blk u)(32) x k(4) x b(8); chunk
        (r, k) is a strided AP (blk and u merge: contiguous layout)."""
    own = rpool.tile([128, 4 * NC], BF16, name=f"own{tagp}", tag=f"own{tagp}")
    hsrc = hout.rearrange("p (s k b) -> p s k b", k=4, b=Bc)
    for k in range(4):
        nc.gpsimd.dma_start(out=own[:, ts(k, NC)].rearrange(
            "p (s b) -> p s b", b=Bc),
            in_=hsrc[:, ts(p, PB * BLK), k, :])
    agt = rpool.tile([128, 2 * PB * BW], BF16, name=f"agt{tagp}",
                     tag=f"agt{tagp}")
    for r in range(2):
        for blk in range(PB):
            base = r * PB * BW + blk * BW
            nc.gpsimd.dma_start(
                out=agt[:, base:base + BW],
                in_=ag_tiles[PB * p + blk][128 * r:128 * (r + 1), :])

    def ag_chunk(r, k):
        base = agt[:, :]
        return bass.AP(base.tensor, base.offset + r * PB * BW + k * Bc,
                       [base.ap[0], [BC4, PB * BLK], [1, Bc]])

    return own, ag_chunk


def _gx2_pass(nc, pools2, p, wx2o_t, wx2g_t, ones, hout1, ag1_tiles, gx2):
    """Fused layer-2 x-projection for pass p: gx2[:, cols] =
    wx2o.T @ [own h1; ones] + wx2g.T @ ag1 (own half zero-weighted)."""
    rpool, psg, spool = pools2
    cols = slice(p * NC, (p + 1) * NC)
    own, ag_chunk = _load_pass_rhs(nc, rpool, "g2", hout1, ag1_tiles, p)
    gdst = gx2.rearrange("p (m f) -> p m f", m=16)
    for q in range(8):
        ps = psg.tile([128, 2 * NC], F32, name="psg", tag="psg")
        for mm in range(2):
            m = q * 2 + mm
            out = ps[:, ts(mm, NC)]
            for k in range(4):  # own h1 (time-aligned)
                nc.tensor.matmul(out, lhsT=wx2o_t[k][:, ts(m, 128)],
                                 rhs=own[:, ts(k, NC)],
                                 start=(k == 0), stop=False)
            nc.tensor.matmul(out, lhsT=wx2o_t[4][:, ts(m, 128)],  # bias
                             rhs=ones[:, :], start=False, stop=False)
            for rk in range(8):  # gathered h1 (own half zero-weighted)
                r, k = rk // 4, rk % 4
                nc.tensor.matmul(out, lhsT=wx2g_t[rk][:, ts(m, 128)],
                                 rhs=ag_chunk(r, k),
                                 start=False, stop=(rk == 7))
        st = spool.tile([128, 2 * NC], BF16, name="g2st", tag="g2st")
        nc.any.tensor_copy(out=st[:, :], in_=ps[:, :])
        nc.gpsimd.dma_start(out=gdst[:, ts(q, 2), cols],
                            in_=st[:, :].rearrange("p (m f) -> p m f", m=2))


def _tag_pass(nc, pools2, p, wtgo_t, wtgg_t, hout2, ag2_tiles, tags):
    """Tag projection for pass p (output cols p*NC..)."""
    rpool, psg, spool = pools2
    cols = slice(p * NC, (p + 1) * NC)
    own, ag_chunk = _load_pass_rhs(nc, rpool, "tg", hout2, ag2_tiles, p)
    ps = psg.tile([64, NC], F32, name="pstag", tag="pstag")
    for k in range(4):
        nc.tensor.matmul(ps[:, :], lhsT=wtgo_t[k][:, :], rhs=own[:, ts(k, NC)],
                         start=(k == 0), stop=False)
    for rk in range(8):
        r, k = rk // 4, rk % 4
        nc.tensor.matmul(ps[:, :], lhsT=wtgg_t[rk][:, :], rhs=ag_chunk(r, k),
                         start=False, stop=(rk == 7))
    st = spool.tile([64, NC], F32, name="tgst", tag="tgst")
    nc.any.tensor_copy(out=st[:, :], in_=ps[:, :])
    nc.gpsimd.dma_start(out=tags[:, cols], in_=st[:, :])


def _gx1_pass(nc, pools2, p, wx1_t, xsT, gx1):
    """Layer-1 x-projection for pass p: gx1[:, cols] = wx1.T @ xsT[:, cols]
    (bias via the ones row of xsT)."""
    rpool, psg, spool = pools2
    cols = slice(p * NC, (p + 1) * NC)
    xt = rpool.tile([128, 5 * NC], BF16, name="g1x", tag="g1x")
    nc.gpsimd.dma_start(
        out=xt[:, :].rearrange("p (k f) -> p k f", k=5),
        in_=xsT[:, cols].rearrange("(k p) f -> p k f", p=128))
    gdst = gx1.rearrange("p (m f) -> p m f", m=16)
    for q in range(8):
        ps = psg.tile([128, 2 * NC], F32, name="psg", tag="psg")
        for mm in range(2):
            m = q * 2 + mm
            out = ps[:, ts(mm, NC)]
            for k in range(5):
                nc.tensor.matmul(out, lhsT=wx1_t[k][:, ts(m, 128)],
                                 rhs=xt[:, ts(k, NC)],
                                 start=(k == 0), stop=(k == 4))
        st = spool.tile([128, 2 * NC], BF16, name="g2st", tag="g2st")
        nc.any.tensor_copy(out=st[:, :], in_=ps[:, :])
        nc.gpsimd.dma_start(out=gdst[:, ts(q, 2), cols],
                            in_=st[:, :].rearrange("p (m f) -> p m f", m=2))


def _build_full():
    nc = bacc.Bacc("TRN2", target_bir_lowering=False, debug=False, num_devices=8)
    xsT = nc.dram_tensor("xsT", [640, N], BF16, kind="ExternalInput")
    wx1 = nc.dram_tensor("wx1", [640, 2048], BF16, kind="ExternalInput")
    wh1 = nc.dram_tensor("wh1", [512, 2048], FP8 if FP8_H else BF16, kind="ExternalInput")
    wx2o = nc.dram_tensor("wx2o", [640, 2048], BF16, kind="ExternalInput")
    wx2g = nc.dram_tensor("wx2g", [1024, 2048], BF16, kind="ExternalInput")
    wh2 = nc.dram_tensor("wh2", [512, 2048], FP8 if FP8_H else BF16, kind="ExternalInput")
    wtgo = nc.dram_tensor("wtgo", [512, 64], BF16, kind="ExternalInput")
    wtgg = nc.dram_tensor("wtgg", [1024, 64], BF16, kind="ExternalInput")
    ident_d = nc.dram_tensor("ident", [128, 128], BF16, kind="ExternalInput")
    tags = nc.dram_tensor("tags", [64, N], F32, kind="ExternalOutput")

    with TileContext(nc) as tc:
        with (
            tc.tile_pool(name="dram", bufs=1, space="DRAM") as dram,
            tc.tile_pool(name="wpool", bufs=1) as wpool,
            tc.tile_pool(name="gxpool", bufs=3) as gxpool,
            tc.tile_pool(name="state", bufs=1) as state,
            tc.tile_pool(name="gbuf", bufs=2) as gbuf,
            tc.tile_pool(name="hring", bufs=3) as hring,
            tc.tile_pool(name="psum", bufs=2, space="PSUM") as psum,
            tc.tile_pool(name="rpool", bufs=2) as rpool,
            tc.tile_pool(name="psg", bufs=2, space="PSUM") as psg,
            tc.tile_pool(name="spool", bufs=2) as spool,
        ):
            ident = wpool.tile([128, 128], BF16, name="ident", tag="ident")
            nc.sync.dma_start(out=ident[:, :], in_=ident_d[:, :])
            ones = wpool.tile([128, NC], BF16, name="ones", tag="ones")
            nc.vector.memset(ones[:, :], 0.0)
            nc.vector.memset(ones[0:1, :], 1.0)
            onesb = wpool.tile([128, BC4], F32, name="onesb", tag="onesb")
            nc.vector.memset(onesb[:, :], 1.0)
            pools = (wpool, gxpool, state, gbuf, hring, psum, ident[:, :],
                     onesb[:, :])
            pools2 = (rpool, psg, spool)

            def loadw(name, src, nk, w=2048):
                out = []
                for k in range(nk):
                    t = wpool.tile([128, w], BF16, name=f"{name}{k}",
                                   tag=f"{name}{k}")
                    nc.sync.dma_start(out=t[:, :], in_=src[ts(k, 128), :])
                    out.append(t)
                return out

            wx1_t = loadw("wx1", wx1, 5)
            wx2o_t = loadw("wx2o", wx2o, 5)
            wx2g_t = loadw("wx2g", wx2g, 8)
            wtgo_t = loadw("wtgo", wtgo, 4, w=64)
            wtgg_t = loadw("wtgg", wtgg, 8, w=64)

            gx1 = dram.tile([128, 16 * N], BF16, name="gx1", tag="gx1")
            # passes 0-1 must precede the scan's first fetches; 2-7 stream
            # into scan1 two blocks ahead of their consumption.
            for p in (0, 1):
                _gx1_pass(nc, pools2, p, wx1_t, xsT[:, :], gx1[:, :])
            gx1_at = {PB * p - 6: p for p in range(2, NPASS)}

            hout1 = dram.tile([128, S * BC4], BF16, name="hout1", tag="hout1")
            hout2 = dram.tile([128, S * BC4], BF16, name="hout2", tag="hout2")
            gx2 = dram.tile([128, 16 * N], BF16, name="gx2", tag="gx2")
            hrev1 = [dram.tile([128, BW], BF16, name=f"hrev1_{b}",
                               tag=f"hrev1_{b}") for b in range(NB)]
            ag1 = [dram.tile([256, BW], BF16, name=f"ag1_{b}",
                             tag=f"ag1_{b}") for b in range(NB)]
            hrev2 = [dram.tile([128, BW], BF16, name=f"hrev2_{b}",
                               tag=f"hrev2_{b}") for b in range(NB)]
            ag2 = [dram.tile([256, BW], BF16, name=f"ag2_{b}",
                             tag=f"ag2_{b}") for b in range(NB)]

            # pass p (blocks PB*p..) is ready after scan block
            # r(p) = max(PB*p+PB-1, NB-1-PB*p); schedule one block later.
            issue_at = {}
            for p in range(NPASS):
                r = max(PB * p + PB - 1, NB - 1 - PB * p)
                issue_at.setdefault(r + 1 if r + 1 < NB else NB, []).append(p)

            def cb1(blk):
                if blk in gx1_at:
                    _gx1_pass(nc, pools2, gx1_at[blk], wx1_t, xsT[:, :],
                              gx1[:, :])
                for p in issue_at.get(blk, []):
                    _gx2_pass(nc, pools2, p, wx2o_t, wx2g_t, ones[:, :],
                              hout1[:, :], ag1, gx2[:, :])

            _scan_cell(nc, tc, pools, wh1, gx1[:, :], hout1[:, :], hrev1, ag1,
                       cb1)
            late = sorted(issue_at.get(NB, []))
            for p in late[:1]:
                _gx2_pass(nc, pools2, p, wx2o_t, wx2g_t, ones[:, :],
                          hout1[:, :], ag1, gx2[:, :])

            def cb2(blk):
                if blk == 2:
                    for p in late[1:]:
                        _gx2_pass(nc, pools2, p, wx2o_t, wx2g_t, ones[:, :],
                                  hout1[:, :], ag1, gx2[:, :])
                for p in issue_at.get(blk, []):
                    _tag_pass(nc, pools2, p, wtgo_t, wtgg_t, hout2[:, :], ag2,
                              tags[:, :])

            _scan_cell(nc, tc, pools, wh2, gx2[:, :], hout2[:, :], hrev2, ag2,
                       cb2)
            for p in sorted(issue_at.get(NB, [])):
                _tag_pass(nc, pools2, p, wtgo_t, wtgg_t, hout2[:, :], ag2,
                          tags[:, :])
    nc.compile()
    return nc


def _bf(x):
    return np.ascontiguousarray(x).astype(ml_dtypes.bfloat16)


def _pack_inputs(words, emb, Wf1, bf1, Wb1, bb1, Wf2, bf2, Wb2, bb2, Wtag):
    words = np.asarray(words).astype(np.int64)
    xs = np.asarray(emb, dtype=np.float32)[words]      # [B, S, E] host gather

    def gates_cat(W, rows):
        return np.concatenate([np.asarray(W[g], dtype=np.float32)[rows]
                               for g in GATE_PERM], axis=1)

    def bias_cat(b):
        return np.concatenate([np.asarray(b[g], dtype=np.float32) for g in GATE_PERM])

    def pack_dir(Wl1, bl1, Wl2, bl2, fwd):
        wx1p = np.zeros((640, 2048), np.float32)
        wx1p[:512] = gates_cat(Wl1, slice(0, 512))
        wx1p[512] = bias_cat(bl1)
        wh1p = gates_cat(Wl1, slice(512, 1024))
        ownsl = slice(0, 512) if fwd else slice(512, 1024)
        wx2m = gates_cat(Wl2, slice(0, 1024))
        wx2op = np.zeros((640, 2048), np.float32)
        wx2op[:512] = wx2m[ownsl]
        wx2op[512] = bias_cat(bl2)
        wx2gp = wx2m.copy()
        wx2gp[ownsl] = 0.0
        wh2p = gates_cat(Wl2, slice(1024, 1536))
        wt = np.asarray(Wtag, dtype=np.float32)
        wtgop = wt[ownsl]
        wtggp = wt.copy()
        wtggp[ownsl] = 0.0
        for wmat in (wx1p, wx2op, wx2gp, wh1p, wh2p):
            wmat[:, 0:512] *= 2.0
        if FP8_H:
            wh1p = np.ascontiguousarray(wh1p * W_SCALE).astype(NPFP8)
            wh2p = np.ascontiguousarray(wh2p * W_SCALE).astype(NPFP8)
            wx1p = wx1p * G_SCALE
            wx2op = wx2op * G_SCALE
            wx2gp = wx2gp * G_SCALE
        else:
            wh1p = _bf(wh1p)
            wh2p = _bf(wh2p)
        return dict(
            wx1=_bf(wx1p), wh1=wh1p,
            wx2o=_bf(wx2op), wx2g=_bf(wx2gp), wh2=wh2p,
            wtgo=_bf(wtgop), wtgg=_bf(wtggp),
        )

    fw = pack_dir(Wf1, bf1, Wf2, bf2, True)
    bw = pack_dir(Wb1, bb1, Wb2, bb2, False)

    def pack_xs(xs_slice):
        # xs_slice [Bc, S, E] -> [640, S*Bc], col = t*Bc + b
        out = np.zeros((640, N), np.float32)
        out[:512] = xs_slice.transpose(2, 1, 0).reshape(512, N)
        out[512] = 1.0
        return _bf(out)

    ins = []
    for core in range(8):
        fwd = core < 4
        bslice = xs[ts(core % 4, Bc)]
        if not fwd:
            bslice = bslice[:, ::-1]
        d = dict(fw if fwd else bw)
        d["xsT"] = pack_xs(bslice)
        d["ident"] = np.eye(128, dtype=ml_dtypes.bfloat16)
        ins.append(d)
    return ins


_NC_CACHE = {}


def _get_nc():
    if "nc" not in _NC_CACHE:
        _NC_CACHE["nc"] = _build_full()
    return _NC_CACHE["nc"]


def kernel(words, lengths, emb, Wf1, bf1, Wb1, bb1, Wf2, bf2, Wb2, bb2, Wtag, btag):
    nc = _get_nc()
    ins = _pack_inputs(words, emb, Wf1, bf1, Wb1, bb1, Wf2, bf2, Wb2, bb2, Wtag)
    res = run_bass_kernel_spmd(nc, ins, core_ids=list(range(8)))
    out = np.empty((B, S, T), np.float32)
    for c in range(4):
        tg = res.results[c]["tags"]               # [64, S*Bc], col = t*Bc+b
        out[ts(c, Bc)] = tg.reshape(T, S, Bc).transpose(2, 1, 0)
    out = out.reshape(B * S, T) + np.asarray(btag, dtype=np.float32)[None, :]
    return np.ascontiguousarray(out.astype(np.float32))

